# revision 1
# baseline (speedup 1.0000x reference)
"""Trainium2 Bass kernel for cross-modal channel-attention fusion (CCDPA).

Math (per batch b):
  pooled[c,m,d] = mean_{w,h} x_m[b,c,d,w,h]
  q = Wq @ pooled[:,0,:] + bq ; k_m = Wk @ pooled[:,m,:] + bk
  a[c,m] = softmax_m(q[c]·k_m[c] / sqrt(D))
  out[b,o,s] = sum_m a[o,m] * (Wc[m] @ x_m[b,:,s] + bc[m,o])
             = sum_m (a[o,m]*Wc[m,o,:]) @ x_m[b,:,s]  + sum_m a[o,m]*bc[m,o]

Sharding: 8 cores = (batch b = p//2) x (d-half = p%2). Each core streams its
64 MiB shard twice: pass 1 reduces to per-(c,m,d) sums; a tiny pairwise
AllGather exchanges the partner's d-half of the pooled sums; attention weights
are computed on-device; pass 2 runs the 4 modality GEMMs with a-scaled weights
accumulated in PSUM.

The 1/(W*H) pooling mean and the 1/sqrt(D) logit scale are folded into the
Wq/Wk weights host-side, and bq/bk ride along as an extra contraction row
(augmented [D+1, D] weight matrices against pooled vectors with an appended
ones-row), so the device math needs no extra scaling ops.
"""

from contextlib import ExitStack

import numpy as np

import concourse.bacc as bacc
import concourse.bass as bass
import concourse.mybir as mybir
import concourse.tile as tile
from concourse.bass_utils import run_bass_kernel_spmd

F32 = mybir.dt.float32

B, C, D, W, H = 4, 256, 32, 32, 32
NCORES = 8
DHALF = D // 2  # d-slices per core
WH = W * H  # spatial elements per d-slice
S = DHALF * WH  # free elements per core shard

# Set to mybir.dt.float32r to run the big GEMMs in fp32r (full PE rate).
MM_DT = mybir.dt.float32


def _emit_program(nc, wh=WH, dhalf=DHALF, mm_dt=MM_DT):
    """Emit the SPMD per-core program. Identical on all 8 cores; per-core
    behavior comes only from per-core input data."""
    f32 = F32
    s = dhalf * wh
    dd = 2 * dhalf  # full D for this (possibly scaled-down) config
    nw = min(512, wh)  # matmul moving-dim chunk
    n_nh = wh // nw
    AX = mybir.AxisListType.X
    AF = mybir.ActivationFunctionType

    xs = [nc.dram_tensor(f"x{m}", [C, s], f32, kind="ExternalInput") for m in range(4)]
    wqT_d = nc.dram_tensor("wqTaug", [dd + 1, dd], f32, kind="ExternalInput")
    wkT_d = nc.dram_tensor("wkTaug", [dd + 1, dd], f32, kind="ExternalInput")
    wc_d = nc.dram_tensor("wc", [4, C, C], f32, kind="ExternalInput")
    bcT_d = nc.dram_tensor("bcT", [C, 4], f32, kind="ExternalInput")
    id_d = nc.dram_tensor("ident", [128, 128], f32, kind="ExternalInput")
    out_d = nc.dram_tensor("out", [C, s], f32, kind="ExternalOutput")

    with tile.TileContext(nc) as tc, ExitStack() as ctx:
        const = ctx.enter_context(tc.tile_pool(name="const", bufs=1))
        pool1 = ctx.enter_context(tc.tile_pool(name="pass1", bufs=16))
        pool2 = ctx.enter_context(tc.tile_pool(name="pass2", bufs=16))
        outp = ctx.enter_context(tc.tile_pool(name="outp", bufs=4))
        attn = ctx.enter_context(tc.tile_pool(name="attn", bufs=1))
        scr = ctx.enter_context(tc.tile_pool(name="scr", bufs=2))
        psA = ctx.enter_context(tc.tile_pool(name="psA", bufs=2, space="PSUM"))
        psM = ctx.enter_context(tc.tile_pool(name="psM", bufs=6, space="PSUM"))
        dramp = ctx.enter_context(tc.tile_pool(name="dramp", bufs=1, space="DRAM"))

        # ---- constant loads (off critical path) ----
        ident = const.tile([128, 128], f32, tag="ident", name="ident")
        nc.sync.dma_start(out=ident[:], in_=id_d[:])
        wqT = const.tile([dd + 1, dd], f32, tag="wqT", name="wqT")
        nc.sync.dma_start(out=wqT[:], in_=wqT_d[:])
        wkT = const.tile([dd + 1, dd], f32, tag="wkT", name="wkT")
        nc.sync.dma_start(out=wkT[:], in_=wkT_d[:])
        wc_sb = []
        for oi in range(2):
            t = const.tile([128, 4 * C], f32, tag=f"wc{oi}", name=f"wc{oi}")
            for m in range(4):
                nc.sync.dma_start(
                    out=t[:, m * C : (m + 1) * C],
                    in_=wc_d[m, oi * 128 : (oi + 1) * 128, :],
                )
            wc_sb.append(t)
        bc_sb = []
        for oi in range(2):
            t = const.tile([128, 4], f32, tag=f"bc{oi}", name=f"bc{oi}")
            nc.sync.dma_start(out=t[:], in_=bcT_d[oi * 128 : (oi + 1) * 128, :])
            bc_sb.append(t)

        # ---- pass 1: pooling sums over (w,h) for each (c, m, d) ----
        # Group structure: each group fills the whole pool once. Before a
        # group's DMAs reuse slots, a sync-engine nop waits for the previous
        # group's last (in-order) DVE reduce, so the per-DMA WAR wait is
        # already covered and each DMA carries at most its 1 allowed
        # embedded wait (walrus DIRECT2D limit).
        praw = [attn.tile([128, 4 * dhalf], f32, tag=f"praw{k}", name=f"praw{k}") for k in range(2)]
        dstep = 1  # d-slices per pass-1 tile (512 KiB DMAs measured fastest)
        for d in range(0, dhalf, dstep):
            for m in range(4):
                for ci in range(2):
                    t = pool1.tile([128, dstep * wh], f32, tag="x1", name="x1")
                    nc.sync.dma_start(
                        out=t[:],
                        in_=xs[m][
                            ci * 128 : (ci + 1) * 128, d * wh : (d + dstep) * wh
                        ],
                    )
                    nc.vector.reduce_sum(
                        out=praw[ci][:, m * dhalf + d : m * dhalf + d + dstep],
                        in_=t[:].rearrange("p (d w) -> p d w", d=dstep),
                        axis=AX,
                    )

        # ---- exchange pooled halves with the partner core ----
        cc_in = dramp.tile([C, 4 * dhalf], f32, tag="cc_in", name="cc_in")
        cc_out = dramp.tile([2 * C, 4 * dhalf], f32, tag="cc_out", name="cc_out")
        for ci in range(2):
            nc.sync.dma_start(
                out=cc_in[ci * 128 : (ci + 1) * 128, :], in_=praw[ci][:]
            )
        nc.gpsimd.collective_compute(
            "AllGather",
            mybir.AluOpType.bypass,
            replica_groups=[[0, 1], [2, 3], [4, 5], [6, 7]],
            ins=[cc_in.opt()],
            outs=[cc_out.opt()],
        )
        # pooled_sb[k][c_local, m*D + d_global]
        pooled = [attn.tile([128, 4 * 2 * dhalf], f32, tag=f"pool{k}", name=f"pool{k}") for k in range(2)]
        for k in range(2):
            for h in range(2):
                for m in range(4):
                    nc.sync.dma_start(
                        out=pooled[k][
                            :, m * 2 * dhalf + h * dhalf : m * 2 * dhalf + (h + 1) * dhalf
                        ],
                        in_=cc_out[
                            h * C + k * 128 : h * C + (k + 1) * 128,
                            m * dhalf : (m + 1) * dhalf,
                        ],
                    )

        # ---- attention weights ----
        # PTaug[m]: [D+1, 256] = pooled sums transposed, plus a ones-row
        ptaug = [attn.tile([dd + 1, C], f32, tag=f"pt{m}", name=f"pt{m}") for m in range(4)]
        for m in range(4):
            nc.vector.memset(ptaug[m][:], 1.0)
            for k in range(2):
                pst = psA.tile([dd, 128], f32, tag="att", name="att")
                nc.tensor.transpose(
                    pst[:], pooled[k][:, m * dd : (m + 1) * dd], ident[:]
                )
                nc.vector.tensor_copy(ptaug[m][0:dd, k * 128 : (k + 1) * 128], pst[:])
        qc = []
        kcs = [[None] * 2 for _ in range(4)]
        for k in range(2):
            psq = psA.tile([128, dd], f32, tag="att", name="att")
            nc.tensor.matmul(
                psq[:], lhsT=ptaug[0][:, k * 128 : (k + 1) * 128], rhs=wqT[:],
                start=True, stop=True,
            )
            t = attn.tile([128, dd], f32, tag=f"qc{k}", name=f"qc{k}")
            nc.vector.tensor_copy(t[:], psq[:])
            qc.append(t)
            for m in range(4):
                psk = psA.tile([128, dd], f32, tag="att", name="att")
                nc.tensor.matmul(
                    psk[:], lhsT=ptaug[m][:, k * 128 : (k + 1) * 128], rhs=wkT[:],
                    start=True, stop=True,
                )
                tk = attn.tile([128, dd], f32, tag=f"kc{m}_{k}", name=f"kc{m}_{k}")
                nc.vector.tensor_copy(tk[:], psk[:])
                kcs[m][k] = tk
        # logits + softmax over m (free dim, 4 wide)
        a_sb = []
        for k in range(2):
            lg = attn.tile([128, 4], f32, tag=f"lg{k}", name=f"lg{k}")
            for m in range(4):
                sc = scr.tile([128, dd], f32, tag="ttr", name="ttr")
                nc.vector.tensor_mul(sc[:], qc[k][:], kcs[m][k][:])
                nc.vector.reduce_sum(out=lg[:, m : m + 1], in_=sc[:], axis=AX)
            mx = attn.tile([128, 1], f32, tag=f"mx{k}", name=f"mx{k}")
            nc.vector.reduce_max(out=mx[:], in_=lg[:], axis=AX)
            nc.vector.tensor_scalar_sub(out=lg[:], in0=lg[:], scalar1=mx[:])
            ex = attn.tile([128, 4], f32, tag=f"ex{k}", name=f"ex{k}")
            nc.scalar.activation(ex[:], lg[:], AF.Exp)
            sm = attn.tile([128, 1], f32, tag=f"sm{k}", name=f"sm{k}")
            nc.vector.reduce_sum(out=sm[:], in_=ex[:], axis=AX)
            rc = attn.tile([128, 1], f32, tag=f"rc{k}", name=f"rc{k}")
            nc.vector.reciprocal(out=rc[:], in_=sm[:])
            at = attn.tile([128, 4], f32, tag=f"a{k}", name=f"a{k}")
            nc.vector.tensor_scalar_mul(out=at[:], in0=ex[:], scalar1=rc[:])
            a_sb.append(at)

        # ---- scaled weights: weff[oi] = a[:,m] * wc rows; wt = weff^T ----
        weff = [attn.tile([128, 4 * C], f32, tag=f"weff{oi}", name=f"weff{oi}") for oi in range(2)]
        beff = []
        for oi in range(2):
            for m in range(4):
                nc.vector.tensor_scalar_mul(
                    out=weff[oi][:, m * C : (m + 1) * C],
                    in0=wc_sb[oi][:, m * C : (m + 1) * C],
                    scalar1=a_sb[oi][:, m : m + 1],
                )
            bt = scr.tile([128, 4], f32, tag="btmp", name="btmp")
            be = attn.tile([128, 1], f32, tag=f"beff{oi}", name=f"beff{oi}")
            nc.vector.tensor_mul(bt[:], a_sb[oi][:], bc_sb[oi][:])
            nc.vector.reduce_sum(out=be[:], in_=bt[:], axis=AX)
            beff.append(be)
        wt_sb = [
            attn.tile([128, 4 * C], mm_dt, tag=f"wt{ci}", name=f"wt{ci}")
            for ci in range(2)
        ]
        for m in range(4):
            for oi in range(2):
                for ci in range(2):
                    psw = psA.tile([128, 128], f32, tag="att", name="att")
                    nc.tensor.transpose(
                        psw[:],
                        weff[oi][:, m * C + ci * 128 : m * C + (ci + 1) * 128],
                        ident[:],
                    )
                    nc.vector.tensor_copy(
                        wt_sb[ci][:, m * C + oi * 128 : m * C + (oi + 1) * 128],
                        psw[:].bitcast(mm_dt),
                    )

        # ---- pass 2: out[o, s] = sum_{m,c} wt[c, o] * x_m[c, s] (+ beff) ----
        for d in range(dhalf):
            xt = {}
            for m in range(4):
                for ci in range(2):
                    t = pool2.tile([128, wh], mm_dt, tag="x2", name="x2")
                    nc.sync.dma_start(
                        out=t[:],
                        in_=xs[m][
                            ci * 128 : (ci + 1) * 128, d * wh : (d + 1) * wh
                        ].bitcast(mm_dt),
                    )
                    xt[(m, ci)] = t
            for oi in range(2):
                ot = outp.tile([128, wh], f32, tag="ot", name="ot")
                for nh in range(n_nh):
                    ps = psM.tile([128, nw], f32, tag="ps", name="ps")
                    for m in range(4):
                        for ci in range(2):
                            nc.tensor.matmul(
                                ps[:],
                                lhsT=wt_sb[ci][
                                    :, m * C + oi * 128 : m * C + (oi + 1) * 128
                                ],
                                rhs=xt[(m, ci)][:, nh * nw : (nh + 1) * nw],
                                start=(m == 0 and ci == 0),
                                stop=(m == 3 and ci == 1),
                            )
                    nc.vector.tensor_scalar_add(
                        out=ot[:, nh * nw : (nh + 1) * nw], in0=ps[:],
                        scalar1=beff[oi][:],
                    )
                # second HWDGE ring (ACT) so the out-DMA's wait doesn't
                # head-of-line-block input prefetch on the Sync ring
                nc.scalar.dma_start(
                    out=out_d[oi * 128 : (oi + 1) * 128, d * wh : (d + 1) * wh],
                    in_=ot[:],
                )
    return nc


_CACHED = {}
LAST_RESULTS = None


def _build(wh=WH, dhalf=DHALF, mm_dt=None):
    if mm_dt is None:
        mm_dt = MM_DT
    key = (wh, dhalf, mm_dt)
    if key not in _CACHED:
        nc = bacc.Bacc(
            "TRN2",
            target_bir_lowering=False,
            debug=False,
            enable_asserts=False,
            num_devices=NCORES,
        )
        _emit_program(nc, wh=wh, dhalf=dhalf, mm_dt=mm_dt)
        nc.compile()
        _CACHED[key] = nc
    return _CACHED[key]


def _host_prep(Wq, bq, Wk, bk, bc, wh_pool, d):
    """Fold pooling mean + logit scale into augmented [D+1, D] q/k weights."""
    scale_q = 1.0 / (wh_pool * np.sqrt(np.float32(d)))
    wqTaug = np.concatenate(
        [(Wq * scale_q).T, (bq / np.sqrt(np.float32(d)))[None, :]], axis=0
    ).astype(np.float32)
    wkTaug = np.concatenate([(Wk / wh_pool).T, bk[None, :]], axis=0).astype(np.float32)
    bcT = np.ascontiguousarray(bc.T).astype(np.float32)
    ident = np.eye(128, dtype=np.float32)
    return wqTaug, wkTaug, bcT, ident


def kernel(m1, m2, m3, m4, Wq, bq, Wk, bk, Wc, bc, **run_kwargs):
    ms = [np.asarray(x, dtype=np.float32) for x in (m1, m2, m3, m4)]
    Wq, bq, Wk, bk, Wc, bc = (
        np.asarray(x, dtype=np.float32) for x in (Wq, bq, Wk, bk, Wc, bc)
    )
    nc = _build()
    wqTaug, wkTaug, bcT, ident = _host_prep(Wq, bq, Wk, bk, bc, WH, D)
    in_maps = []
    for p in range(NCORES):
        b, h = divmod(p, 2)
        im = {
            f"x{m}": np.ascontiguousarray(
                ms[m][b, :, h * DHALF : (h + 1) * DHALF]
            ).reshape(C, S)
            for m in range(4)
        }
        im.update(wqTaug=wqTaug, wkTaug=wkTaug, wc=Wc, bcT=bcT, ident=ident)
        in_maps.append(im)
    global LAST_RESULTS
    res = run_bass_kernel_spmd(
        nc, in_maps, core_ids=list(range(NCORES)), **run_kwargs
    )
    LAST_RESULTS = res
    out = np.empty((B, C, D, W, H), np.float32)
    for p in range(NCORES):
        b, h = divmod(p, 2)
        out[b, :, h * DHALF : (h + 1) * DHALF] = res.results[p]["out"].reshape(
            C, DHALF, W, H
        )
    return out



# revision 36
# speedup vs baseline: 1.8518x; 1.8518x over previous
"""Trainium2 Bass kernel for cross-modal channel-attention fusion (CCDPA).

Math (per batch b):
  pooled[c,m,d] = mean_{w,h} x_m[b,c,d,w,h]
  q = Wq @ pooled[:,0,:] + bq ; k_m = Wk @ pooled[:,m,:] + bk
  a[c,m] = softmax_m(q[c]·k_m[c] / sqrt(D))
  out[b,o,s] = sum_m a[o,m] * (Wc[m] @ x_m[b,:,s] + bc[m,o])
             = sum_m (a[o,m]*Wc[m,o,:]) @ x_m[b,:,s]  + sum_m a[o,m]*bc[m,o]

Sharding/schedule: 8 cores, 4 pipelined phases (= the 4 batches). In phase t
every core reads ITS dpc=D/8 d-slices of batch t once (16 MiB fp32), reduces
them to pooled sums on DVE, and converts them to a bf16 SBUF cache on ACT.
The per-core pooled partials are exchanged with one 8-way AllGather (16 KiB),
attention weights are computed on-device, the conv weights are a-scaled, and
the batch's GEMMs run from the bf16 cache (no second HBM read). Phase t+1's
reads stream while phase t's attention+GEMM run, so HBM traffic is the floor:
64 MiB read + 16 MiB written per core.

Emission is software-pipelined: phase t's GEMM + PSUM drains are emitted
after phase t+1's read/reduce/convert section, so the in-order DVE stream
never blocks the next phase's pooling behind drain work.

The 1/(W*H) pooling mean and the 1/sqrt(D) logit scale are folded into the
Wq/Wk weights host-side, and bq/bk ride along as an extra contraction row
(augmented [D+1, D] weight matrices against pooled-sum vectors with an
appended ones-row).
"""

from contextlib import ExitStack

import numpy as np

import concourse.bacc as bacc
import concourse.bass as bass
import concourse.mybir as mybir
import concourse.tile as tile
from concourse.bass_utils import run_bass_kernel_spmd

F32 = mybir.dt.float32
BF16 = mybir.dt.bfloat16

B, C, D, W, H = 4, 256, 32, 32, 32
NCORES = 8
M = 4  # modalities
CI = 2  # 128-row halves of C
T = B  # phases == batches
DPC = D // NCORES  # d-slices per core per phase
WH = W * H


def _emit_program(nc, wh=WH, dpc=DPC, stage=3):
    f32 = F32
    s = dpc * wh  # free elems per (m, ci) cache tile
    dd = dpc * NCORES  # full D for this (possibly scaled-down) config
    fr = dpc * CI * M  # praw free width
    chunk = min(s, 2048)  # staging DMA chunk (1 MiB at full size)
    nch = s // chunk
    dlpc = max(1, chunk // wh)  # d-slices per staging chunk
    AX = mybir.AxisListType.X
    AF = mybir.ActivationFunctionType

    xs = [
        nc.dram_tensor(f"x{m}", [T * C, s], f32, kind="ExternalInput")
        for m in range(M)
    ]
    wqT_d = nc.dram_tensor("wqTaug", [dd + 1, dd], f32, kind="ExternalInput")
    wkT_d = nc.dram_tensor("wkTaug", [dd + 1, dd], f32, kind="ExternalInput")
    # wcP[o, m*C + c] = Wc[m, o, c]
    wc_d = nc.dram_tensor("wcP", [C, M * C], f32, kind="ExternalInput")
    bct_d = nc.dram_tensor("bcT", [C, M], f32, kind="ExternalInput")
    id_d = nc.dram_tensor("ident", [128, 128], f32, kind="ExternalInput")
    out_d = nc.dram_tensor("out", [T * C, s], f32, kind="ExternalOutput")

    with tile.TileContext(nc) as tc, ExitStack() as ctx:
        const = ctx.enter_context(tc.tile_pool(name="const", bufs=1))
        stgp = ctx.enter_context(tc.tile_pool(name="stg", bufs=3))
        cachep = ctx.enter_context(tc.tile_pool(name="cache", bufs=2))
        outp = ctx.enter_context(tc.tile_pool(name="outp", bufs=2))
        attn = ctx.enter_context(tc.tile_pool(name="attn", bufs=2))
        wpool = ctx.enter_context(tc.tile_pool(name="wpool", bufs=1))
        scrp = ctx.enter_context(tc.tile_pool(name="scr", bufs=2))
        psA = ctx.enter_context(tc.tile_pool(name="psA", bufs=2, space="PSUM"))
        psM = ctx.enter_context(tc.tile_pool(name="psM", bufs=6, space="PSUM"))
        dramp = ctx.enter_context(tc.tile_pool(name="dramp", bufs=2, space="DRAM"))

        # ---- constants (off critical path) ----
        ident = const.tile([128, 128], f32, tag="ident", name="ident")
        nc.sync.dma_start(out=ident[:], in_=id_d[:])
        wq_sb = const.tile([dd + 1, dd], f32, tag="wqT", name="wqT")
        nc.sync.dma_start(out=wq_sb[:], in_=wqT_d[:])
        wk_sb = const.tile([dd + 1, dd], f32, tag="wkT", name="wkT")
        nc.sync.dma_start(out=wk_sb[:], in_=wkT_d[:])
        wc_sb = []
        for oi in range(CI):
            t_ = const.tile([128, M * C], f32, tag=f"wc{oi}", name=f"wc{oi}")
            nc.sync.dma_start(out=t_[:], in_=wc_d[oi * 128 : (oi + 1) * 128, :])
            wc_sb.append(t_)
        bct_sb = []
        for k in range(CI):
            t_ = const.tile([128, M], f32, tag=f"bct{k}", name=f"bct{k}")
            nc.sync.dma_start(out=t_[:], in_=bct_d[k * 128 : (k + 1) * 128, :])
            bct_sb.append(t_)

        nw = min(512, wh)  # PSUM bank limit: 512 fp32 per partition
        n_nh = wh // nw
        pending = None  # (t, cache, wt, beff) awaiting GEMM emission

        def emit_gemm(t, cache, wt, beff):
            for dl in range(dpc):
                for oi in range(CI):
                    ot = outp.tile([128, wh], f32, tag="ot", name="ot")
                    for nh in range(n_nh):
                        ps = psM.tile([128, nw], f32, tag="ps", name="ps")
                        off = dl * wh + nh * nw
                        for m in range(M):
                            for ci in range(CI):
                                nc.tensor.matmul(
                                    ps[:],
                                    lhsT=wt[ci][
                                        :, m * C + oi * 128 : m * C + (oi + 1) * 128
                                    ],
                                    rhs=cache[(m, ci)][:, off : off + nw],
                                    start=(m == 0 and ci == 0),
                                    stop=(m == M - 1 and ci == CI - 1),
                                )
                        nc.vector.tensor_scalar_add(
                            out=ot[:, nh * nw : (nh + 1) * nw],
                            in0=ps[:],
                            scalar1=beff[oi][:],
                        )
                    nc.scalar.dma_start(
                        out=out_d[
                            t * C + oi * 128 : t * C + (oi + 1) * 128,
                            dl * wh : (dl + 1) * wh,
                        ],
                        in_=ot[:],
                    )

        for t in range(T):
            # ---- reads + pooling sums + bf16 cache conversion ----
            praw = attn.tile([128, fr], f32, tag="praw", name="praw")
            prv = praw[:].rearrange("p (dl m ci) -> p dl m ci", dl=dpc, m=M, ci=CI)
            cache = {}
            for m in range(M):
                for ci in range(CI):
                    ct = cachep.tile([128, s], BF16, tag=f"c{m}{ci}", name=f"c{m}{ci}")
                    cache[(m, ci)] = ct
                    for j in range(nch):
                        stg = stgp.tile([128, chunk], f32, tag="stg", name="stg")
                        nc.sync.dma_start(
                            out=stg[:],
                            in_=xs[m][
                                t * C + ci * 128 : t * C + (ci + 1) * 128,
                                j * chunk : (j + 1) * chunk,
                            ],
                        )
                        nc.vector.reduce_sum(
                            out=prv[:, j * dlpc : (j + 1) * dlpc, m, ci],
                            in_=stg[:].rearrange("p (dl w) -> p dl w", dl=dlpc),
                            axis=AX,
                        )
                        nc.scalar.activation(
                            ct[:, j * chunk : (j + 1) * chunk], stg[:], AF.Copy
                        )

            # ---- deferred GEMM of the previous phase ----
            if pending is not None:
                emit_gemm(*pending)
                pending = None

            if stage < 2:
                nc.sync.dma_start(out=out_d[t * C : t * C + 128, 0:fr], in_=praw[:])
                continue

            # ---- exchange pooled sums (transpose -> 8-way AllGather) ----
            pst = psA.tile([fr, 128], f32, tag="att", name="att")
            nc.tensor.transpose(pst[:], praw[:], ident[:])
            trT = attn.tile([fr, 128], f32, tag="trT", name="trT")
            nc.vector.tensor_copy(trT[:], pst[:])
            cc_in = dramp.tile([dpc, M * CI * 128], f32, tag="cc_in", name="cc_in")
            cc_out = dramp.tile(
                [NCORES * dpc, M * CI * 128], f32, tag="cc_out", name="cc_out"
            )
            nc.scalar.dma_start(
                out=cc_in[:].rearrange("dl (m ci c) -> (dl m ci) c", m=M, ci=CI),
                in_=trT[:],
            )
            if stage < 3:  # debug: skip collective, fake gather with local data
                for h in range(NCORES):
                    nc.scalar.dma_start(
                        out=cc_out[h * dpc : (h + 1) * dpc, :], in_=cc_in[:]
                    )
            else:
                nc.gpsimd.collective_compute(
                    "AllGather",
                    mybir.AluOpType.bypass,
                    replica_groups=[list(range(NCORES))],
                    ins=[cc_in.opt()],
                    outs=[cc_out.opt()],
                )
            # ptA[d, m*256 + ci*128 + c] = pooled_sum[c, m, d]; ones row at d=dd
            ptA = wpool.tile([dd + 1, M * CI * 128], f32, tag="ptA", name="ptA")
            nc.vector.memset(ptA[:], 1.0)  # row dd stays 1.0 (bias ones-row)
            nc.scalar.dma_start(out=ptA[0:dd, :], in_=cc_out[:])

            # ---- attention weights a[o, m] (o-halves k) ----
            a_sb, beff = [], []
            for k in range(CI):
                psq = psA.tile([128, dd], f32, tag="att", name="att")
                nc.tensor.matmul(
                    psq[:], lhsT=ptA[:, k * 128 : (k + 1) * 128], rhs=wq_sb[:],
                    start=True, stop=True,
                )
                q_sb = attn.tile([128, dd], f32, tag=f"q{k}", name=f"q{k}")
                nc.vector.tensor_copy(q_sb[:], psq[:])
                lg = attn.tile([128, M], f32, tag=f"lg{k}", name=f"lg{k}")
                for m in range(M):
                    psk = psA.tile([128, dd], f32, tag="att", name="att")
                    nc.tensor.matmul(
                        psk[:],
                        lhsT=ptA[:, m * C + k * 128 : m * C + (k + 1) * 128],
                        rhs=wk_sb[:],
                        start=True, stop=True,
                    )
                    scr = scrp.tile([128, dd], f32, tag="scr", name="scr")
                    nc.vector.tensor_mul(scr[:], q_sb[:], psk[:])
                    nc.vector.reduce_sum(out=lg[:, m : m + 1], in_=scr[:], axis=AX)
                mx = attn.tile([128, 1], f32, tag=f"mx{k}", name=f"mx{k}")
                nc.vector.reduce_max(out=mx[:], in_=lg[:], axis=AX)
                nc.vector.tensor_scalar_sub(out=lg[:], in0=lg[:], scalar1=mx[:])
                ex = attn.tile([128, M], f32, tag=f"ex{k}", name=f"ex{k}")
                nc.scalar.activation(ex[:], lg[:], AF.Exp)
                sm = attn.tile([128, 1], f32, tag=f"sm{k}", name=f"sm{k}")
                nc.vector.reduce_sum(out=sm[:], in_=ex[:], axis=AX)
                rc = attn.tile([128, 1], f32, tag=f"rc{k}", name=f"rc{k}")
                nc.vector.reciprocal(out=rc[:], in_=sm[:])
                at = attn.tile([128, M], f32, tag=f"a{k}", name=f"a{k}")
                nc.vector.tensor_scalar_mul(out=at[:], in0=ex[:], scalar1=rc[:])
                a_sb.append(at)
                scb = scrp.tile([128, M], f32, tag="scb", name="scb")
                be = attn.tile([128, 1], f32, tag=f"be{k}", name=f"be{k}")
                nc.vector.tensor_mul(scb[:], at[:], bct_sb[k][:])
                nc.vector.reduce_sum(out=be[:], in_=scb[:], axis=AX)
                beff.append(be)

            # ---- a-scaled weights: weff[oi] = a[:,m] * wc rows; wt = weff^T ----
            weff = [
                wpool.tile([128, M * C], f32, tag=f"weff{oi}", name=f"weff{oi}")
                for oi in range(CI)
            ]
            for oi in range(CI):
                for m in range(M):
                    nc.vector.tensor_scalar_mul(
                        out=weff[oi][:, m * C : (m + 1) * C],
                        in0=wc_sb[oi][:, m * C : (m + 1) * C],
                        scalar1=a_sb[oi][:, m : m + 1],
                    )
            wt = [
                wpool.tile([128, M * C], BF16, tag=f"wt{ci}", name=f"wt{ci}")
                for ci in range(CI)
            ]
            for m in range(M):
                for oi in range(CI):
                    for ci in range(CI):
                        psw = psA.tile([128, 128], f32, tag="att", name="att")
                        nc.tensor.transpose(
                            psw[:],
                            weff[oi][:, m * C + ci * 128 : m * C + (ci + 1) * 128],
                            ident[:],
                        )
                        nc.vector.tensor_copy(
                            wt[ci][:, m * C + oi * 128 : m * C + (oi + 1) * 128],
                            psw[:],
                        )

            pending = (t, cache, wt, beff)

        if pending is not None:
            emit_gemm(*pending)
            pending = None
    return nc


_CACHED = {}
LAST_RESULTS = None


def _build(wh=WH, dpc=DPC, stage=3):
    key = (wh, dpc, stage)
    if key not in _CACHED:
        nc = bacc.Bacc(
            "TRN2",
            target_bir_lowering=False,
            debug=False,
            enable_asserts=False,
            num_devices=NCORES,
        )
        _emit_program(nc, wh=wh, dpc=dpc, stage=stage)
        nc.compile()
        _CACHED[key] = nc
    return _CACHED[key]


def _host_prep(Wq, bq, Wk, bk, Wc, bc, wh_pool, d):
    """Fold pooling mean + logit scale into augmented [D+1, D] q/k weights;
    permute Wc to [o, m*C + c]."""
    scale_q = 1.0 / (wh_pool * np.sqrt(np.float32(d)))
    wqTaug = np.concatenate(
        [(Wq * scale_q).T, (bq / np.sqrt(np.float32(d)))[None, :]], axis=0
    ).astype(np.float32)
    wkTaug = np.concatenate([(Wk / wh_pool).T, bk[None, :]], axis=0).astype(np.float32)
    # wcP[o, m*C + c] = Wc[m, o, c]
    wcP = np.ascontiguousarray(
        Wc.transpose(1, 0, 2).reshape(Wc.shape[1], -1)
    ).astype(np.float32)
    bcT = np.ascontiguousarray(bc.T).astype(np.float32)
    ident = np.eye(128, dtype=np.float32)
    return wqTaug, wkTaug, wcP, bcT, ident


def _make_in_maps(ms, Wq, bq, Wk, bk, Wc, bc, wh_pool, dpc):
    d = dpc * NCORES
    wqTaug, wkTaug, wcP, bcT, ident = _host_prep(Wq, bq, Wk, bk, Wc, bc, wh_pool, d)
    in_maps = []
    for p in range(NCORES):
        im = {
            f"x{m}": np.ascontiguousarray(
                ms[m][:, :, p * dpc : (p + 1) * dpc]
            ).reshape(T * C, dpc * wh_pool)
            for m in range(M)
        }
        im.update(wqTaug=wqTaug, wkTaug=wkTaug, wcP=wcP, bcT=bcT, ident=ident)
        in_maps.append(im)
    return in_maps


def kernel(m1, m2, m3, m4, Wq, bq, Wk, bk, Wc, bc, **run_kwargs):
    ms = [np.asarray(x, dtype=np.float32) for x in (m1, m2, m3, m4)]
    Wq, bq, Wk, bk, Wc, bc = (
        np.asarray(x, dtype=np.float32) for x in (Wq, bq, Wk, bk, Wc, bc)
    )
    nc = _build()
    in_maps = _make_in_maps(ms, Wq, bq, Wk, bk, Wc, bc, WH, DPC)
    global LAST_RESULTS
    res = run_bass_kernel_spmd(
        nc, in_maps, core_ids=list(range(NCORES)), **run_kwargs
    )
    LAST_RESULTS = res
    out = np.empty((B, C, D, W, H), np.float32)
    for p in range(NCORES):
        out[:, :, p * DPC : (p + 1) * DPC] = res.results[p]["out"].reshape(
            B, C, DPC, W, H
        )
    return out


# revision 43
# speedup vs baseline: 2.0824x; 1.1245x over previous
"""Trainium2 Bass kernel for cross-modal channel-attention fusion (CCDPA).

Math (per batch b):
  pooled[c,m,d] = mean_{w,h} x_m[b,c,d,w,h]
  q = Wq @ pooled[:,0,:] + bq ; k_m = Wk @ pooled[:,m,:] + bk
  a[c,m] = softmax_m(q[c]·k_m[c] / sqrt(D))
  out[b,o,s] = sum_m a[o,m] * (Wc[m] @ x_m[b,:,s] + bc[m,o])
             = sum_m (a[o,m]*Wc[m,o,:]) @ x_m[b,:,s]  + sum_m a[o,m]*bc[m,o]

Sharding/schedule: 8 cores, 4 pipelined phases (= the 4 batches). In phase t
every core reads ITS dpc=D/8 d-slices of batch t once (16 MiB fp32); the ACT
engine converts them to a bf16 SBUF cache and produces the pooling sums as a
free side effect (activation Copy with accum_out). The per-core pooled
partials are exchanged with one 8-way AllGather (16 KiB) launched purely by
DMA (SBUF DMA-transpose + scatter, no compute engine), attention weights are
computed on-device, the conv weights are a-scaled, and the batch's GEMMs run
from the bf16 cache (no second HBM read). HBM traffic is the floor: 64 MiB
read + 16 MiB written per core.

Emission is software-pipelined one phase deep: iteration t emits
[attention-tail(t-1), reads(t), AG-launch(t), GEMM(t-1)] so the in-order
engine streams never block the read pipeline on collective latency.

The 1/(W*H) pooling mean and the 1/sqrt(D) logit scale are folded into the
Wq/Wk weights host-side, and bq/bk ride along as an extra contraction row
(augmented [D+1, D] weight matrices against pooled-sum vectors with an
appended ones-row).
"""

from contextlib import ExitStack

import numpy as np

import concourse.bacc as bacc
import concourse.bass as bass
import concourse.mybir as mybir
import concourse.tile as tile
from concourse.bass_utils import run_bass_kernel_spmd

F32 = mybir.dt.float32
BF16 = mybir.dt.bfloat16

B, C, D, W, H = 4, 256, 32, 32, 32
NCORES = 8
M = 4  # modalities
CI = 2  # 128-row halves of C
T = B  # phases == batches
DPC = D // NCORES  # d-slices per core per phase
WH = W * H


def _emit_program(nc, wh=WH, dpc=DPC, stage=3):
    f32 = F32
    s = dpc * wh  # free elems per (m, ci) cache tile
    dd = dpc * NCORES  # full D for this (possibly scaled-down) config
    fr = dpc * CI * M  # praw free width
    chunk = min(s, 2048)  # staging DMA chunk (1 MiB at full size)
    nch = s // chunk
    dlpc = max(1, chunk // wh)  # d-slices per staging chunk
    AX = mybir.AxisListType.X
    AF = mybir.ActivationFunctionType

    xs = [
        nc.dram_tensor(f"x{m}", [T * C, s], f32, kind="ExternalInput")
        for m in range(M)
    ]
    wqT_d = nc.dram_tensor("wqTaug", [dd + 1, dd], f32, kind="ExternalInput")
    wkT_d = nc.dram_tensor("wkTaug", [dd + 1, dd], f32, kind="ExternalInput")
    # wcP[o, m*C + c] = Wc[m, o, c]
    wc_d = nc.dram_tensor("wcP", [C, M * C], f32, kind="ExternalInput")
    bct_d = nc.dram_tensor("bcT", [C, M], f32, kind="ExternalInput")
    id_d = nc.dram_tensor("ident", [128, 128], f32, kind="ExternalInput")
    out_d = nc.dram_tensor("out", [T * C, s], f32, kind="ExternalOutput")

    with tile.TileContext(nc) as tc, ExitStack() as ctx:
        const = ctx.enter_context(tc.tile_pool(name="const", bufs=1))
        stgp = ctx.enter_context(tc.tile_pool(name="stg", bufs=3))
        cachep = ctx.enter_context(tc.tile_pool(name="cache", bufs=2))
        outp = ctx.enter_context(tc.tile_pool(name="outp", bufs=2))
        attn = ctx.enter_context(tc.tile_pool(name="attn", bufs=2))
        wpool = ctx.enter_context(tc.tile_pool(name="wpool", bufs=1))
        scrp = ctx.enter_context(tc.tile_pool(name="scr", bufs=2))
        psA = ctx.enter_context(tc.tile_pool(name="psA", bufs=2, space="PSUM"))
        psM = ctx.enter_context(tc.tile_pool(name="psM", bufs=6, space="PSUM"))
        dramp = ctx.enter_context(tc.tile_pool(name="dramp", bufs=2, space="DRAM"))

        # ---- constants (off critical path) ----
        ident = const.tile([128, 128], f32, tag="ident", name="ident")
        nc.sync.dma_start(out=ident[:], in_=id_d[:])
        wqf = const.tile([dd + 1, dd], f32, tag="wqTf", name="wqTf")
        nc.sync.dma_start(out=wqf[:], in_=wqT_d[:])
        wq_sb = const.tile([dd + 1, dd], BF16, tag="wqT", name="wqT")
        nc.scalar.activation(wq_sb[:], wqf[:], mybir.ActivationFunctionType.Copy)
        wkf = const.tile([dd + 1, dd], f32, tag="wkTf", name="wkTf")
        nc.sync.dma_start(out=wkf[:], in_=wkT_d[:])
        wk_sb = const.tile([dd + 1, dd], BF16, tag="wkT", name="wkT")
        nc.scalar.activation(wk_sb[:], wkf[:], mybir.ActivationFunctionType.Copy)
        wc_sb = []
        for oi in range(CI):
            t_ = const.tile([128, M * C], f32, tag=f"wc{oi}", name=f"wc{oi}")
            nc.sync.dma_start(out=t_[:], in_=wc_d[oi * 128 : (oi + 1) * 128, :])
            wc_sb.append(t_)
        bct_sb = []
        for k in range(CI):
            t_ = const.tile([128, M], f32, tag=f"bct{k}", name=f"bct{k}")
            nc.sync.dma_start(out=t_[:], in_=bct_d[k * 128 : (k + 1) * 128, :])
            bct_sb.append(t_)

        nw = min(512, wh)  # PSUM bank limit: 512 fp32 per partition
        n_nh = wh // nw

        def emit_reads(t):
            """Stream phase t's x; ACT converts to bf16 cache with pooled
            sums accumulated as a side effect (accum_out)."""
            praw = attn.tile([128, fr], f32, tag="praw", name="praw")
            cache = {}
            for m in range(M):
                for ci in range(CI):
                    ct = cachep.tile([128, s], BF16, tag=f"c{m}{ci}", name=f"c{m}{ci}")
                    cache[(m, ci)] = ct
                    for j in range(nch):
                        stg = stgp.tile([128, chunk], f32, tag="stg", name="stg")
                        nc.sync.dma_start(
                            out=stg[:],
                            in_=xs[m][
                                t * C + ci * 128 : t * C + (ci + 1) * 128,
                                j * chunk : (j + 1) * chunk,
                            ],
                        )
                        for u in range(dlpc):
                            dl = j * dlpc + u
                            col = (dl * M + m) * CI + ci
                            nc.scalar.activation(
                                ct[:, dl * wh : (dl + 1) * wh],
                                stg[:, u * wh : (u + 1) * wh],
                                AF.Copy,
                                accum_out=praw[:, col : col + 1],
                            )
            return praw, cache

        def emit_ag_launch(t, praw):
            """Pooled-sum exchange, no compute engines: SBUF DMA-transpose,
            scatter into cc_in, 8-way AllGather."""
            pst = psA.tile([fr, 128], f32, tag="att", name="att")
            nc.tensor.transpose(pst[:], praw[:], ident[:])
            trT = attn.tile([fr, 128], BF16, tag="trT", name="trT")
            nc.vector.tensor_copy(trT[:], pst[:])
            cc_in = dramp.tile([dpc, M * CI * 128], BF16, tag="cc_in", name="cc_in")
            cc_out = dramp.tile(
                [NCORES * dpc, M * CI * 128], BF16, tag="cc_out", name="cc_out"
            )
            nc.sync.dma_start(
                out=cc_in[:].rearrange("dl (m ci c) -> (dl m ci) c", m=M, ci=CI),
                in_=trT[:],
            )
            if stage < 3:  # debug: skip collective, fake gather with local data
                for h in range(NCORES):
                    nc.sync.dma_start(
                        out=cc_out[h * dpc : (h + 1) * dpc, :], in_=cc_in[:]
                    )
            else:
                nc.gpsimd.collective_compute(
                    "AllGather",
                    mybir.AluOpType.bypass,
                    replica_groups=[list(range(NCORES))],
                    ins=[cc_in.opt()],
                    outs=[cc_out.opt()],
                )
            return cc_out

        def emit_attn_tail(t, cc_out):
            """ptA load, q/k matmuls, softmax, a-scaled transposed weights."""
            # ptA[d, m*256 + ci*128 + c] = pooled_sum[c, m, d]; ones row at dd
            ptA = wpool.tile([dd + 1, M * CI * 128], BF16, tag="ptA", name="ptA")
            nc.vector.memset(ptA[:], 1.0)  # row dd stays 1.0 (bias ones-row)
            nc.scalar.dma_start(out=ptA[0:dd, :], in_=cc_out[:])

            a_sb, beff = [], []
            for k in range(CI):
                psq = psA.tile([128, dd], f32, tag="att", name="att")
                nc.tensor.matmul(
                    psq[:], lhsT=ptA[:, k * 128 : (k + 1) * 128], rhs=wq_sb[:],
                    start=True, stop=True,
                )
                q_sb = attn.tile([128, dd], f32, tag=f"q{k}", name=f"q{k}")
                nc.vector.tensor_copy(q_sb[:], psq[:])
                lg = attn.tile([128, M], f32, tag=f"lg{k}", name=f"lg{k}")
                for m in range(M):
                    psk = psA.tile([128, dd], f32, tag="att", name="att")
                    nc.tensor.matmul(
                        psk[:],
                        lhsT=ptA[:, m * C + k * 128 : m * C + (k + 1) * 128],
                        rhs=wk_sb[:],
                        start=True, stop=True,
                    )
                    scr = scrp.tile([128, dd], f32, tag="scr", name="scr")
                    nc.vector.tensor_mul(scr[:], q_sb[:], psk[:])
                    nc.vector.reduce_sum(out=lg[:, m : m + 1], in_=scr[:], axis=AX)
                mx = attn.tile([128, 1], f32, tag=f"mx{k}", name=f"mx{k}")
                nc.vector.reduce_max(out=mx[:], in_=lg[:], axis=AX)
                nc.vector.tensor_scalar_sub(out=lg[:], in0=lg[:], scalar1=mx[:])
                ex = attn.tile([128, M], f32, tag=f"ex{k}", name=f"ex{k}")
                nc.scalar.activation(ex[:], lg[:], AF.Exp)
                sm = attn.tile([128, 1], f32, tag=f"sm{k}", name=f"sm{k}")
                nc.vector.reduce_sum(out=sm[:], in_=ex[:], axis=AX)
                rc = attn.tile([128, 1], f32, tag=f"rc{k}", name=f"rc{k}")
                nc.vector.reciprocal(out=rc[:], in_=sm[:])
                at = attn.tile([128, M], f32, tag=f"a{k}", name=f"a{k}")
                nc.vector.tensor_scalar_mul(out=at[:], in0=ex[:], scalar1=rc[:])
                a_sb.append(at)
                scb = scrp.tile([128, M], f32, tag="scb", name="scb")
                be = attn.tile([128, 1], f32, tag=f"be{k}", name=f"be{k}")
                nc.vector.tensor_mul(scb[:], at[:], bct_sb[k][:])
                nc.vector.reduce_sum(out=be[:], in_=scb[:], axis=AX)
                beff.append(be)

            # weff[oi] = a[:,m] * wc rows; wt[ci] = weff^T (bf16)
            weff = [
                wpool.tile([128, M * C], f32, tag=f"weff{oi}", name=f"weff{oi}")
                for oi in range(CI)
            ]
            for oi in range(CI):
                for m in range(M):
                    nc.vector.tensor_scalar_mul(
                        out=weff[oi][:, m * C : (m + 1) * C],
                        in0=wc_sb[oi][:, m * C : (m + 1) * C],
                        scalar1=a_sb[oi][:, m : m + 1],
                    )
            wt = [
                wpool.tile([128, M * C], BF16, tag=f"wt{ci}", name=f"wt{ci}")
                for ci in range(CI)
            ]
            for m in range(M):
                for oi in range(CI):
                    for ci in range(CI):
                        psw = psA.tile([128, 128], f32, tag="att", name="att")
                        nc.tensor.transpose(
                            psw[:],
                            weff[oi][:, m * C + ci * 128 : m * C + (ci + 1) * 128],
                            ident[:],
                        )
                        nc.vector.tensor_copy(
                            wt[ci][:, m * C + oi * 128 : m * C + (oi + 1) * 128],
                            psw[:],
                        )
            return wt, beff

        def emit_gemm(t, cache, wt, beff):
            for dl in range(dpc):
                for oi in range(CI):
                    ot = outp.tile([128, wh], f32, tag="ot", name="ot")
                    for nh in range(n_nh):
                        ps = psM.tile([128, nw], f32, tag="ps", name="ps")
                        off = dl * wh + nh * nw
                        for m in range(M):
                            for ci in range(CI):
                                nc.tensor.matmul(
                                    ps[:],
                                    lhsT=wt[ci][
                                        :, m * C + oi * 128 : m * C + (oi + 1) * 128
                                    ],
                                    rhs=cache[(m, ci)][:, off : off + nw],
                                    start=(m == 0 and ci == 0),
                                    stop=(m == M - 1 and ci == CI - 1),
                                )
                        nc.vector.tensor_scalar_add(
                            out=ot[:, nh * nw : (nh + 1) * nw],
                            in0=ps[:],
                            scalar1=beff[oi][:],
                        )
                    nc.scalar.dma_start(
                        out=out_d[
                            t * C + oi * 128 : t * C + (oi + 1) * 128,
                            dl * wh : (dl + 1) * wh,
                        ],
                        in_=ot[:],
                    )

        pending = None  # (t, cache, cc_out) awaiting attention tail + GEMM
        for t in range(T):
            if pending is not None:
                wt, beff = emit_attn_tail(pending[0], pending[2])
            praw, cache = emit_reads(t)
            cc_out = emit_ag_launch(t, praw)
            if pending is not None:
                emit_gemm(pending[0], pending[1], wt, beff)
            pending = (t, cache, cc_out)
        wt, beff = emit_attn_tail(pending[0], pending[2])
        emit_gemm(pending[0], pending[1], wt, beff)
    return nc


_CACHED = {}
LAST_RESULTS = None


def _build(wh=WH, dpc=DPC, stage=3):
    key = (wh, dpc, stage)
    if key not in _CACHED:
        nc = bacc.Bacc(
            "TRN2",
            target_bir_lowering=False,
            debug=False,
            enable_asserts=False,
            num_devices=NCORES,
        )
        _emit_program(nc, wh=wh, dpc=dpc, stage=stage)
        nc.compile()
        _CACHED[key] = nc
    return _CACHED[key]


def _host_prep(Wq, bq, Wk, bk, Wc, bc, wh_pool, d):
    """Fold pooling mean + logit scale into augmented [D+1, D] q/k weights;
    permute Wc to [o, m*C + c]."""
    scale_q = 1.0 / (wh_pool * np.sqrt(np.float32(d)))
    wqTaug = np.concatenate(
        [(Wq * scale_q).T, (bq / np.sqrt(np.float32(d)))[None, :]], axis=0
    ).astype(np.float32)
    wkTaug = np.concatenate([(Wk / wh_pool).T, bk[None, :]], axis=0).astype(np.float32)
    wcP = np.ascontiguousarray(
        Wc.transpose(1, 0, 2).reshape(Wc.shape[1], -1)
    ).astype(np.float32)
    bcT = np.ascontiguousarray(bc.T).astype(np.float32)
    ident = np.eye(128, dtype=np.float32)
    return wqTaug, wkTaug, wcP, bcT, ident


def _make_in_maps(ms, Wq, bq, Wk, bk, Wc, bc, wh_pool, dpc):
    d = dpc * NCORES
    wqTaug, wkTaug, wcP, bcT, ident = _host_prep(Wq, bq, Wk, bk, Wc, bc, wh_pool, d)
    in_maps = []
    for p in range(NCORES):
        im = {
            f"x{m}": np.ascontiguousarray(
                ms[m][:, :, p * dpc : (p + 1) * dpc]
            ).reshape(T * C, dpc * wh_pool)
            for m in range(M)
        }
        im.update(wqTaug=wqTaug, wkTaug=wkTaug, wcP=wcP, bcT=bcT, ident=ident)
        in_maps.append(im)
    return in_maps


def kernel(m1, m2, m3, m4, Wq, bq, Wk, bk, Wc, bc, **run_kwargs):
    ms = [np.asarray(x, dtype=np.float32) for x in (m1, m2, m3, m4)]
    Wq, bq, Wk, bk, Wc, bc = (
        np.asarray(x, dtype=np.float32) for x in (Wq, bq, Wk, bk, Wc, bc)
    )
    nc = _build()
    in_maps = _make_in_maps(ms, Wq, bq, Wk, bk, Wc, bc, WH, DPC)
    global LAST_RESULTS
    res = run_bass_kernel_spmd(
        nc, in_maps, core_ids=list(range(NCORES)), **run_kwargs
    )
    LAST_RESULTS = res
    out = np.empty((B, C, D, W, H), np.float32)
    for p in range(NCORES):
        out[:, :, p * DPC : (p + 1) * DPC] = res.results[p]["out"].reshape(
            B, C, DPC, W, H
        )
    return out
